# revision 10
# baseline (speedup 1.0000x reference)
"""Concordance-index (C-index) kernel for Trainium2, 8 NeuronCores — v3.

Math
----
Reference computes, over all pairs i<j of N=16384 samples:
    cc = ((y_i>=y_j & yh_i>=yh_j & st_j) | (y_i<=y_j & yh_i<=yh_j & st_i)) & triu
    tp = ((y_i<=y_j & st_i) | (y_i>=y_j & st_j)) & triu
    out = sum(cc) / sum(tp)

Columns with st_j = 0 contribute nothing, so the sweep is N x ns over
(all i) x (event j): with A = [y_i >= y_j], B = [yh_i >= yh_j],
    sum(cc) = S1 - ns,  S1 = sum_{i, j in E} A*B   (diag = 1 each)
    sum(tp) = S2 - ns,  S2 = sum_{i, j in E} A

v3 key idea: the host permutes the i-axis to y-sorted order and packs
event columns y-sorted + rank-interleaved across cores.  Then per
128-event j-group, A is a step function along i, and most [128, 4096]
i-tiles are fully decided:
    "zero" tile (all y_i < all y_j): contributes nothing -> SKIPPED
    "ones" tile (all y_i >= all y_j): A==1, so only sum(B) is needed ->
        one DVE ts-accum (or ScalarE sign-accum) per tile
    "mixed" tile (~1.3 per group): full compute, flavored A (DVE
        ts-plain a01 + stt fused b*a with row-accum; PE counts a01) or
        B (2 ScalarE Signs + DVE tt product + PE ones-matmul) to
        balance DVE/ACT/PE.
This cuts tile-ops from 32 full to ~11 mixed + ~9 cheap per core.
The (pattern, flavors) are data-dependent -> compiled per pattern and
cached. Pads use y_j=-BIG / yh_j=+BIG so every formula contributes an
exact host-known constant. bf16 tie noise ~5e-4 total, gate is 2e-2.
"""

import math
import os
import sys

import numpy as np

for _p in ("/opt/trn_rl_repo", "/root/.axon_site", "/root/.axon_site/_ro/trn_rl_repo"):
    if os.path.isdir(_p) and _p not in sys.path:
        sys.path.append(_p)

import ml_dtypes

import concourse.bacc as bacc
import concourse.mybir as mybir
from concourse import bass_utils
from concourse import tile

N = 16384
P = 128
NCORES = 8
F = 4096                 # i-tile width (free axis)
IT = N // F              # 4 i-tiles
BIG = np.float32(1e30)

FP32 = mybir.dt.float32
BF16 = mybir.dt.bfloat16
Alu = mybir.AluOpType
ActF = mybir.ActivationFunctionType

# tile classes
ZERO, ONES_DVE, ONES_ACT, MIX_A, MIX_B = 0, 1, 2, 3, 4


def _bf(x):
    return np.asarray(x, np.float32).astype(ml_dtypes.bfloat16).astype(np.float32)


class Shard:
    """Host-side packing: y-sorted i-axis, rank-interleaved event slots,
    per-(group, itile) class pattern."""

    def __init__(self, y, yh, status):
        y = np.asarray(y, np.float32)
        yh = np.asarray(yh, np.float32)
        ybf = _bf(y)
        yhbf = _bf(yh)
        # i-axis permutation: stable sort by bf16 y
        self.idx = np.argsort(ybf, kind="stable")
        self.y_sorted = ybf[self.idx]          # fed as y_full
        self.yh_perm = yhbf[self.idx]          # fed as yh_full
        pos_of_orig = np.empty(N, np.int64)
        pos_of_orig[self.idx] = np.arange(N)

        ev = np.nonzero(np.asarray(status) == 1)[0]
        self.ns = len(ev)
        # events sorted by bf16 y, rank r -> core r % NCORES
        ev_sorted = ev[np.argsort(ybf[ev], kind="stable")]
        self.jt_e = max(1, math.ceil(self.ns / (NCORES * P)))
        per = self.jt_e * P
        self.per = per
        self.nt = IT * self.jt_e

        # slot tables per core: orig index (-1 = pad), y/yh scalar values
        self.slot_orig = np.full((NCORES, per), -1, np.int64)
        self.y_sl = np.full((NCORES, per), -BIG, np.float32)
        self.yh_sl = np.full((NCORES, per), BIG, np.float32)
        for c in range(NCORES):
            mine = ev_sorted[c::NCORES]
            k = len(mine)
            self.slot_orig[c, :k] = mine
            self.y_sl[c, :k] = ybf[mine]
            self.yh_sl[c, :k] = yhbf[mine]

        # crossing positions: first sorted-i with y_i >= y_j
        # (exact: both sides bf16 values)
        ssl = np.searchsorted(self.y_sorted, self.y_sl.reshape(-1),
                              side="left").reshape(NCORES, per)

        # per-(group, itile) class, unioned across cores
        cls = np.zeros((self.jt_e, IT), np.int64)
        self.group_lo = np.zeros((NCORES, self.jt_e), np.int64)
        self.group_hi = np.zeros((NCORES, self.jt_e), np.int64)
        for g in range(self.jt_e):
            lo_all, hi_all = N, 0
            for c in range(NCORES):
                sl = slice(g * P, (g + 1) * P)
                real = self.slot_orig[c, sl] >= 0
                if real.any():
                    lo = int(ssl[c, sl][real].min())
                    hi = int(ssl[c, sl][real].max())
                else:
                    lo, hi = N, 0   # all-pad group: everything "zero"
                self.group_lo[c, g] = lo
                self.group_hi[c, g] = hi
                lo_all = min(lo_all, lo)
                hi_all = max(hi_all, hi)
            for it in range(IT):
                t0, t1 = it * F, (it + 1) * F
                if t1 <= lo_all:
                    cls[g, it] = ZERO
                elif t0 >= hi_all:
                    cls[g, it] = ONES_DVE
                else:
                    cls[g, it] = MIX_A
        self.cls = cls
        self._assign_flavors()
        # event diag position (sorted-i space) per core/slot
        self.pos_of_orig = pos_of_orig
        self.pattern = tuple(cls.reshape(-1).tolist())

    def _assign_flavors(self):
        """Balance DVE / ACT / PE by greedy assignment (measured ns/tile)."""
        cost = {MIX_A: (6000, 0, 5200), MIX_B: (2600, 9900, 5200),
                ONES_DVE: (4800, 0, 0), ONES_ACT: (0, 4980, 0)}
        load = [0.0, 0.0, 0.0]
        order = [(g, it) for g in range(self.jt_e) for it in range(IT)]
        # mixed first (larger), then ones
        for kind, opts in ((MIX_A, (MIX_A, MIX_B)),
                           (ONES_DVE, (ONES_DVE, ONES_ACT))):
            for g, it in order:
                if self.cls[g, it] != kind:
                    continue
                best, bestmax = None, None
                for o in opts:
                    trial = [load[k] + cost[o][k] for k in range(3)]
                    m = max(trial)
                    if bestmax is None or m < bestmax:
                        bestmax, best = m, o
                self.cls[g, it] = best
                for k in range(3):
                    load[k] += cost[best][k]
        self.load = load


def build_bass(shard):
    jt_e, nt, cls = shard.jt_e, shard.nt, shard.cls
    nc = bacc.Bacc(debug=False, num_devices=NCORES)

    y_full = nc.dram_tensor("y_full", [1, N], BF16, kind="ExternalInput")
    yh_full = nc.dram_tensor("yh_full", [1, N], BF16, kind="ExternalInput")
    y_sl = nc.dram_tensor("y_sl", [P, jt_e], FP32, kind="ExternalInput")
    yh_sl = nc.dram_tensor("yh_sl", [P, jt_e], FP32, kind="ExternalInput")
    o_r = nc.dram_tensor("o_r", [P, nt], FP32, kind="ExternalOutput")
    o_g = nc.dram_tensor("o_g", [P, nt], FP32, kind="ExternalOutput")
    o_h = nc.dram_tensor("o_h", [P, nt], FP32, kind="ExternalOutput")
    o_b = nc.dram_tensor("o_b", [P, nt], FP32, kind="ExternalOutput")
    o_pa = nc.dram_tensor("o_pa", [1, 512], FP32, kind="ExternalOutput")
    o_pp = nc.dram_tensor("o_pp", [1, 512], FP32, kind="ExternalOutput")

    n_mm = {"pa": int((cls == MIX_A).sum()) * (F // 512),
            "pp": int((cls == MIX_B).sum()) * (F // 512)}
    # which i-tiles are needed at all
    it_used = [it for it in range(IT)
               if any(cls[g, it] != ZERO for g in range(jt_e))]

    with tile.TileContext(nc) as tc:
        with (
            tc.tile_pool(name="const", bufs=1) as cpool,
            tc.tile_pool(name="bcast", bufs=2) as bpool,
            tc.tile_pool(name="work", bufs=3) as wpool,
            tc.tile_pool(name="psum", bufs=1, space="PSUM") as ppool,
        ):
            y_j = cpool.tile([P, jt_e], FP32)
            nc.sync.dma_start(out=y_j[:, :], in_=y_sl[:, :])
            yh_j = cpool.tile([P, jt_e], FP32)
            nc.sync.dma_start(out=yh_j[:, :], in_=yh_sl[:, :])
            neg_y = cpool.tile([P, jt_e], FP32)
            nc.vector.tensor_scalar_mul(neg_y[:, :], y_j[:, :], -1.0)
            neg_yh = cpool.tile([P, jt_e], FP32)
            nc.vector.tensor_scalar_mul(neg_yh[:, :], yh_j[:, :], -1.0)

            ones_w = cpool.tile([P, 1], BF16)
            nc.vector.memset(ones_w[:, :], 1.0)

            accs = {}
            for nm in ("r", "g", "h", "b"):
                # no memset: combine() only reads columns their class's
                # accum op writes (accum_out overwrites, not adds)
                t = cpool.tile([P, nt], FP32, tag=f"acc_{nm}")
                accs[nm] = t
            acc_pa = ppool.tile([1, 512], FP32)
            acc_pp = ppool.tile([1, 512], FP32)
            seen = {"pa": 0, "pp": 0}

            def pe_reduce(key, acc, src):
                for ch in range(F // 512):
                    seen[key] += 1
                    nc.tensor.matmul(
                        acc[0:1, 0:512],
                        ones_w[:, :],
                        src[:, ch * 512:(ch + 1) * 512],
                        start=(seen[key] == 1),
                        stop=(seen[key] == n_mm[key]),
                    )

            for it in it_used:
                need_y = any(cls[g, it] in (MIX_A, MIX_B) for g in range(jt_e))
                yib = None
                if need_y:
                    yib = bpool.tile([P, F], BF16, tag="yib")
                    nc.sync.dma_start(
                        out=yib[:, :],
                        in_=y_full[0:1, it * F:(it + 1) * F].to_broadcast((P, F)),
                    )
                yhib = bpool.tile([P, F], BF16, tag="yhib")
                nc.sync.dma_start(
                    out=yhib[:, :],
                    in_=yh_full[0:1, it * F:(it + 1) * F].to_broadcast((P, F)),
                )
                for g in range(jt_e):
                    col = it * jt_e + g
                    k = cls[g, it]
                    if k == ZERO:
                        continue
                    if k == ONES_DVE:
                        b01 = wpool.tile([P, F], BF16, tag="stt_out")
                        nc.vector.tensor_scalar(
                            out=b01[:, :], in0=yhib[:, :],
                            scalar1=yh_j[:, g:g + 1], scalar2=0.0,
                            op0=Alu.is_ge, op1=Alu.add,
                            accum_out=accs["b"][:, col:col + 1],
                        )
                    elif k == ONES_ACT:
                        hs = wpool.tile([P, F], BF16, tag="hs")
                        nc.scalar.activation(
                            out=hs[:, :], in_=yhib[:, :], func=ActF.Sign,
                            bias=neg_yh[:, g:g + 1], scale=1.0,
                            accum_out=accs["h"][:, col:col + 1],
                        )
                    elif k == MIX_A:
                        a01 = wpool.tile([P, F], BF16, tag="a01")
                        nc.vector.tensor_scalar(
                            out=a01[:, :], in0=yib[:, :],
                            scalar1=y_j[:, g:g + 1], scalar2=None,
                            op0=Alu.is_ge,
                        )
                        pab = wpool.tile([P, F], BF16, tag="stt_out")
                        nc.vector.scalar_tensor_tensor(
                            out=pab[:, :], in0=yhib[:, :],
                            scalar=yh_j[:, g:g + 1], in1=a01[:, :],
                            op0=Alu.is_ge, op1=Alu.mult,
                            accum_out=accs["r"][:, col:col + 1],
                        )
                        pe_reduce("pa", acc_pa, a01)
                    else:  # MIX_B
                        gs = wpool.tile([P, F], BF16, tag="gs")
                        nc.scalar.activation(
                            out=gs[:, :], in_=yib[:, :], func=ActF.Sign,
                            bias=neg_y[:, g:g + 1], scale=1.0,
                            accum_out=accs["g"][:, col:col + 1],
                        )
                        hs = wpool.tile([P, F], BF16, tag="hs")
                        nc.scalar.activation(
                            out=hs[:, :], in_=yhib[:, :], func=ActF.Sign,
                            bias=neg_yh[:, g:g + 1], scale=1.0,
                            accum_out=accs["h"][:, col:col + 1],
                        )
                        p = wpool.tile([P, F], BF16, tag="p")
                        nc.vector.tensor_tensor(
                            out=p[:, :], in0=gs[:, :], in1=hs[:, :],
                            op=Alu.mult)
                        pe_reduce("pp", acc_pp, p)

            nc.sync.dma_start(out=o_r[:, :], in_=accs["r"][:, :])
            nc.sync.dma_start(out=o_g[:, :], in_=accs["g"][:, :])
            nc.sync.dma_start(out=o_h[:, :], in_=accs["h"][:, :])
            nc.sync.dma_start(out=o_b[:, :], in_=accs["b"][:, :])
            for acc, o, key in ((acc_pa, o_pa, "pa"), (acc_pp, o_pp, "pp")):
                stg = cpool.tile([1, 512], FP32, tag=f"stg_{o.name}")
                if n_mm[key] == 0:
                    nc.vector.memset(stg[:, :], 0.0)
                else:
                    nc.vector.tensor_copy(out=stg[:, :], in_=acc[0:1, 0:512])
                nc.sync.dma_start(out=o[:, :], in_=stg[:, :])

    nc.compile()
    return nc


_NC_CACHE = {}


def _get_nc(shard):
    key = (shard.jt_e, shard.pattern)
    if key not in _NC_CACHE:
        _NC_CACHE[key] = build_bass(shard)
    return _NC_CACHE[key]


def make_in_maps(shard):
    y2 = np.ascontiguousarray(
        shard.y_sorted.astype(ml_dtypes.bfloat16).reshape(1, N))
    yh2 = np.ascontiguousarray(
        shard.yh_perm.astype(ml_dtypes.bfloat16).reshape(1, N))
    in_maps = []
    for c in range(NCORES):
        in_maps.append({
            "y_full": y2,
            "yh_full": yh2,
            # slot s = g*P + p  ->  [p, g]
            "y_sl": np.ascontiguousarray(
                shard.y_sl[c].reshape(shard.jt_e, P).T),
            "yh_sl": np.ascontiguousarray(
                shard.yh_sl[c].reshape(shard.jt_e, P).T),
        })
    return in_maps


def combine(results, shard):
    """Exact reconstruction in float64 from device partial sums."""
    jt_e, nt, cls = shard.jt_e, shard.nt, shard.cls
    Ff = float(F)
    S1 = 0.0
    S2 = 0.0
    n_pad = (shard.slot_orig < 0).sum(axis=1)  # per core (in last group)
    for c, r in enumerate(results):
        rr = r["o_r"].astype(np.float64)
        gg = r["o_g"].astype(np.float64)
        hh = r["o_h"].astype(np.float64)
        bb = r["o_b"].astype(np.float64)
        pa = float(r["o_pa"].astype(np.float64).sum())
        pp = float(r["o_pp"].astype(np.float64).sum())
        real = (shard.slot_orig[c] >= 0).reshape(jt_e, P)  # [g, p]
        S2 += pa  # pad pollution removed below
        for g in range(jt_e):
            nreal = int(real[g].sum())
            npad = P - nreal
            for it in range(IT):
                col = it * jt_e + g
                k = cls[g, it]
                if k == ZERO:
                    continue
                if k == ONES_DVE:
                    # A==1 for real slots: S1 += sum(B), S2 += F per real
                    S1 += float(bb[:, col][real[g]].sum())
                    S2 += Ff * nreal
                elif k == ONES_ACT:
                    # sum(B) ~= (F + sum(h_sign))/2 per real partition
                    S1 += float(
                        (Ff + hh[:, col][real[g]]).sum()) / 2.0
                    S2 += Ff * nreal
                elif k == MIX_A:
                    S1 += float(rr[:, col].sum())   # pads contribute 0
                    S2 -= Ff * npad                  # pad a01==1 rows in pa
                else:  # MIX_B
                    # per-cell (1+g)(1+h)/4 identity: pads cancel exactly
                    S1 += (Ff * P + float(gg[:, col].sum())
                           + float(hh[:, col].sum())) / 4.0
                    # (F + sum g)/2 per partition; pad rows give F -> remove
                    S2 += (Ff * P + float(gg[:, col].sum())) / 2.0 - Ff * npad
        # gh product term of the MIX_B tiles (PSUM-accumulated per core)
        S1 += pp / 4.0
    # diagonal corrections for MIX_B tiles (est 1/4 resp 1/2, want 1)
    for c in range(NCORES):
        for s in range(shard.per):
            o = shard.slot_orig[c, s]
            if o < 0:
                continue
            g = s // P
            it = int(shard.pos_of_orig[o]) // F
            if cls[g, it] == MIX_B:
                S1 += 0.75
                S2 += 0.5
    ns = float(shard.ns)
    c32 = np.float32(S1 - ns)
    t32 = np.float32(S2 - ns)
    return np.asarray(np.float32(c32 / t32))


def kernel(y, y_hat, status, _run_kwargs=None):
    shard = Shard(y, y_hat, status)
    nc = _get_nc(shard)
    in_maps = make_in_maps(shard)
    kw = dict(_run_kwargs or {})
    res = bass_utils.run_bass_kernel_spmd(
        nc, in_maps, core_ids=list(range(NCORES)), **kw)
    out = combine(res.results, shard)
    if _run_kwargs is not None:
        return out, res
    return out


if __name__ == "__main__":
    rng = np.random.default_rng(0)
    y = rng.standard_normal(N).astype(np.float32)
    yh = rng.standard_normal(N).astype(np.float32)
    st = (rng.integers(0, 2, N)).astype(np.int32)
    print(kernel(y, yh, st))
